# revision 27
# baseline (speedup 1.0000x reference)
"""Deformable-attention encoder layer (single level) on 8 Trainium2 cores.

Data-parallel over batch: B=16 -> 2 images per NeuronCore. Each core runs the
full layer for its 2 images; no collectives.

MSDA sampling strategy: the value projection is computed 4 times with the src
rows shifted by (0, 1, 64, 65), giving, for every spatial cell e=(y,x), the
full 2x2 bilinear patch [v(y,x), v(y,x+1), v(y+1,x), v(y+1,x+1)] laid out
contiguously per head (128 bf16 = 256B). These patches are written to a DRAM
table indexed by (e, head), and fetched with gpsimd dma_gather in chunks of
1024 indices (>=2048 indices per gather overflows the SWDGE descriptor ring
on this runtime and wedges the device; per-partition-offset indirect DMAs
work but cost ~1us of Q7 time each, 2048 of them dominated the old runtime).
The int16 index tile must be in dma_gather's wrapped layout (index i at
partition i%16, column i//16, replicated across the 8 Q7 core groups), which
is produced by a PE transpose + a shuffled DRAM round trip. Bilinear corner
weights, OOB masks and attention weights are folded into 16 per-point
coefficients applied on the Vector engine, followed by a strided reduce.
"""

import sys
from contextlib import ExitStack

import numpy as np

sys.path.insert(0, "/opt/trn_rl_repo")

import concourse.bass as bass
import concourse.bacc as bacc
import concourse.mybir as mybir
import concourse.tile as tile
from concourse.bass_utils import run_bass_kernel_spmd

F32 = mybir.dt.float32
BF16 = mybir.dt.bfloat16
I16 = mybir.dt.int16
I32 = mybir.dt.int32
AF = mybir.ActivationFunctionType
OP = mybir.AluOpType

B, LQ, C = 16, 4096, 256
NH, NP, DH = 8, 4, 32
HS, WS = 64, 64
DFF = 1024
NCORES = 8
BPC = B // NCORES          # batches per core
P = 128
NT = LQ // P               # q-tiles per batch
EPAD = 128                 # srcT column padding for shifted windows

_CACHE = {}


def _bcast_ap(handle_or_ap, n_free):
    """DRAM [n] -> AP [128, n] replicated across partitions."""
    ap = handle_or_ap[:] if not isinstance(handle_or_ap, bass.AP) else handle_or_ap
    return bass.AP(tensor=ap.tensor, offset=ap.offset, ap=[[0, P], [1, n_free]])


def _expand_d(ap, d):
    """Append a 0-step broadcast dim of size d to an AP."""
    return bass.AP(tensor=ap.tensor, offset=ap.offset, ap=[*ap.ap, [0, d]])


def build_nc():
    nc = bacc.Bacc("TRN2")

    src_d = nc.dram_tensor("src", [BPC * LQ, C], F32, kind="ExternalInput")
    # host-pretransposed [batch*C, LQ] copies: feed the matmul lhsT layout
    # directly, removing all phase-A PE transposes
    srcT_d = nc.dram_tensor("srcT", [BPC * C, LQ], BF16, kind="ExternalInput")
    posT_d = nc.dram_tensor("posT", [BPC * C, LQ], BF16, kind="ExternalInput")
    ref_d = nc.dram_tensor("refp", [BPC * LQ, 2], F32, kind="ExternalInput")
    w_off_d = nc.dram_tensor("W_off", [C, 64], BF16, kind="ExternalInput")
    b_off_d = nc.dram_tensor("b_off", [64], F32, kind="ExternalInput")
    w_attn_d = nc.dram_tensor("W_attn", [C, 32], BF16, kind="ExternalInput")
    b_attn_d = nc.dram_tensor("b_attn", [32], F32, kind="ExternalInput")
    w_val_d = nc.dram_tensor("W_val", [C, C], BF16, kind="ExternalInput")
    b_val_d = nc.dram_tensor("b_val", [C], F32, kind="ExternalInput")
    w_out_d = nc.dram_tensor("W_out", [C, C], BF16, kind="ExternalInput")
    b_out_d = nc.dram_tensor("b_out", [C], F32, kind="ExternalInput")
    ln1g_d = nc.dram_tensor("ln1_g", [C], F32, kind="ExternalInput")
    ln1b_d = nc.dram_tensor("ln1_b", [C], F32, kind="ExternalInput")
    w1_d = nc.dram_tensor("W1", [C, DFF], BF16, kind="ExternalInput")
    b1_d = nc.dram_tensor("b1", [DFF], F32, kind="ExternalInput")
    w2_d = nc.dram_tensor("W2", [DFF, C], BF16, kind="ExternalInput")
    b2_d = nc.dram_tensor("b2", [C], F32, kind="ExternalInput")
    ln2g_d = nc.dram_tensor("ln2_g", [C], F32, kind="ExternalInput")
    ln2b_d = nc.dram_tensor("ln2_b", [C], F32, kind="ExternalInput")
    out_d = nc.dram_tensor("out", [BPC * LQ, C], F32, kind="ExternalOutput")

    # patch table: row (b*4096 + e)*8 + h ... laid out [b, e, h, 128], bf16
    h2_d = nc.dram_tensor("h2tab", [BPC * LQ * NH, 4 * DH], BF16, kind="Internal")
    # per-tile gather indices in dma_gather's wrapped layout, int16:
    # block (b, t) holds idx(q, j) at offset j*8 + (q%16)*256 + q//16
    idxshuf_d = nc.dram_tensor("idxshuf", [BPC * NT * P * 32], I16,
                               kind="Internal")

    ident_d = nc.inline_tensor(np.eye(P, dtype=np.float32), "identconst")
    # per-(h,p) head index constant, replicated over partitions
    hc = np.tile(np.repeat(np.arange(NH, dtype=np.float32), NP)[None, :], (P, 1))
    hc_d = nc.inline_tensor(hc, "hconst")

    with ExitStack() as ctx:
        tc = ctx.enter_context(tile.TileContext(nc))
        consts = ctx.enter_context(tc.tile_pool(name="consts", bufs=1))
        persist = ctx.enter_context(tc.tile_pool(name="persist", bufs=1))
        work = ctx.enter_context(tc.tile_pool(name="work", bufs=2))
        io = ctx.enter_context(tc.tile_pool(name="io", bufs=3))
        patches_p = ctx.enter_context(tc.tile_pool(name="patches", bufs=4))
        ppt = ctx.enter_context(tc.tile_pool(name="ppt", bufs=2, space="PSUM"))
        pps = ctx.enter_context(tc.tile_pool(name="pps", bufs=4, space="PSUM"))
        ppf = ctx.enter_context(tc.tile_pool(name="ppf", bufs=2, space="PSUM"))

        # ---- constants / weights into SBUF ----
        ident = consts.tile([P, P], F32)
        nc.sync.dma_start(out=ident[:], in_=ident_d[:, :])
        hcst = consts.tile([P, 32], F32)
        nc.sync.dma_start(out=hcst[:], in_=hc_d[:, :])

        wb = consts.tile([P, 2, 96], BF16)          # W_off|W_attn, 2 k-tiles
        for k in range(2):
            nc.sync.dma_start(out=wb[:, k, 0:64], in_=w_off_d[128 * k:128 * (k + 1), :])
            nc.sync.dma_start(out=wb[:, k, 64:96], in_=w_attn_d[128 * k:128 * (k + 1), :])
        wval = consts.tile([P, 2, C], BF16)
        wout = consts.tile([P, 2, C], BF16)
        for k in range(2):
            nc.sync.dma_start(out=wval[:, k, :], in_=w_val_d[128 * k:128 * (k + 1), :])
            nc.sync.dma_start(out=wout[:, k, :], in_=w_out_d[128 * k:128 * (k + 1), :])
        w1 = consts.tile([P, 2, DFF], BF16)
        for k in range(2):
            nc.sync.dma_start(out=w1[:, k, :], in_=w1_d[128 * k:128 * (k + 1), :])
        w2 = consts.tile([P, 8, C], BF16)
        for j in range(8):
            nc.sync.dma_start(out=w2[:, j, :], in_=w2_d[128 * j:128 * (j + 1), :])

        boffattn = consts.tile([P, 96], F32)
        nc.sync.dma_start(out=boffattn[:, 0:64], in_=_bcast_ap(b_off_d, 64))
        nc.sync.dma_start(out=boffattn[:, 64:96], in_=_bcast_ap(b_attn_d, 32))
        bval = consts.tile([P, C], F32)
        nc.sync.dma_start(out=bval[:], in_=_bcast_ap(b_val_d, C))
        bout = consts.tile([P, C], F32)
        nc.sync.dma_start(out=bout[:], in_=_bcast_ap(b_out_d, C))
        ln1g = consts.tile([P, C], F32)
        nc.sync.dma_start(out=ln1g[:], in_=_bcast_ap(ln1g_d, C))
        ln1b = consts.tile([P, C], F32)
        nc.sync.dma_start(out=ln1b[:], in_=_bcast_ap(ln1b_d, C))
        b2r = consts.tile([P, C], F32)
        nc.sync.dma_start(out=b2r[:], in_=_bcast_ap(b2_d, C))
        ln2g = consts.tile([P, C], F32)
        nc.sync.dma_start(out=ln2g[:], in_=_bcast_ap(ln2g_d, C))
        ln2b = consts.tile([P, C], F32)
        nc.sync.dma_start(out=ln2b[:], in_=_bcast_ap(ln2b_d, C))
        # b1 transposed: [128, 8] with b1t[p, j] = b1[128j + p]
        b1t = consts.tile([P, 8], F32)
        b1_ap = bass.AP(tensor=b1_d[:].tensor, offset=0, ap=[[1, P], [P, 8]])
        nc.sync.dma_start(out=b1t[:], in_=b1_ap)

        for b in range(BPC):
            # ============ PHASE A: projections, idx/weights, patch table ====
            srcT = persist.tile([P, 2, LQ + EPAD], BF16, tag="srcT")
            for k in range(2):
                nc.vector.memset(srcT[:, k, LQ:LQ + EPAD], 0.0)
            # per-batch tags: batch b+1's phase A can start while batch b's
            # phase B still reads these
            idx_all = persist.tile([P, NT, 256], I16, tag=f"idx_all{b}")
            w4_all = persist.tile([P, NT, NH, NP, 2, 2], BF16, tag=f"w4_all{b}")

            # A1: load the pretransposed src for this batch (shifted windows
            # in the value projection read up to 192 columns ahead).
            for k in range(2):
                nc.sync.dma_start(
                    out=srcT[:, k, 0:LQ],
                    in_=srcT_d[b * 2 * P + k * P:b * 2 * P + (k + 1) * P, :])

            for t in range(NT):
                r0 = b * LQ + t * P
                ref_t = io.tile([P, 2], F32, tag="ref_t")
                nc.sync.dma_start(out=ref_t[:], in_=ref_d[r0:r0 + P, :])

                posT = work.tile([P, 2, P], BF16, tag="posT")
                for k in range(2):
                    nc.sync.dma_start(
                        out=posT[:, k, :],
                        in_=bass.AP(tensor=posT_d[:, :].tensor,
                                    offset=(b * 2 * P + k * P) * LQ + t * P,
                                    ap=[[LQ, P], [1, P]]))

                # off|attn projection: q @ [W_off|W_attn] = (src+pos) @ Wb
                oa = pps.tile([P, 96], F32, tag="ps")
                nc.tensor.matmul(oa[:], srcT[:, 0, t * P:(t + 1) * P], wb[:, 0, :],
                                 start=True, stop=False)
                nc.tensor.matmul(oa[:], posT[:, 0, :], wb[:, 0, :], start=False, stop=False)
                nc.tensor.matmul(oa[:], srcT[:, 1, t * P:(t + 1) * P], wb[:, 1, :],
                                 start=False, stop=False)
                nc.tensor.matmul(oa[:], posT[:, 1, :], wb[:, 1, :], start=False, stop=True)

                # attention softmax over NP (logits are small; skip max-sub)
                aw_e = work.tile([P, 32], F32, tag="aw_e")
                nc.vector.tensor_tensor(out=aw_e[:], in0=oa[:, 64:96], in1=boffattn[:, 64:96],
                                        op=OP.add)
                nc.scalar.activation(out=aw_e[:], in_=aw_e[:], func=AF.Exp)
                aw_s = work.tile([P, 8], F32, tag="aw_s")
                nc.vector.reduce_sum(out=aw_s[:], in_=aw_e[:].rearrange("p (h q) -> p h q", h=8),
                                     axis=mybir.AxisListType.X)
                nc.vector.reciprocal(out=aw_s[:], in_=aw_s[:])
                aw = work.tile([P, 32], F32, tag="aw")
                nc.vector.tensor_tensor(
                    out=aw[:].rearrange("p (h q) -> p h q", h=8),
                    in0=aw_e[:].rearrange("p (h q) -> p h q", h=8),
                    in1=_expand_d(aw_s[:], NP), op=OP.mult)

                # sampling locations: px = off + b_off + (ref*64 - 0.5)
                refs = work.tile([P, 2], F32, tag="refs")
                nc.vector.tensor_scalar(out=refs[:], in0=ref_t[:], scalar1=64.0,
                                        scalar2=-0.5, op0=OP.mult, op1=OP.add)
                px = work.tile([P, 64], F32, tag="px")
                nc.vector.tensor_tensor(out=px[:], in0=oa[:, 0:64], in1=boffattn[:, 0:64],
                                        op=OP.add)
                refs_b = bass.AP(tensor=refs[:].tensor, offset=refs[:].offset,
                                 ap=[refs[:].ap[0], [0, 32], [1, 2]])
                nc.vector.tensor_tensor(out=px[:].rearrange("p (a c) -> p a c", c=2),
                                        in0=px[:].rearrange("p (a c) -> p a c", c=2),
                                        in1=refs_b, op=OP.add)
                # clip to [-2, 66], bias by 1024, floor via int cast + fixup
                pc = work.tile([P, 64], F32, tag="pc")
                nc.vector.tensor_scalar(out=pc[:], in0=px[:], scalar1=-2.0, scalar2=66.0,
                                        op0=OP.max, op1=OP.min)
                pb = work.tile([P, 64], F32, tag="pb")
                nc.vector.tensor_scalar_add(pb[:], pc[:], 1024.0)
                pi = work.tile([P, 64], I32, tag="pi")
                nc.vector.tensor_copy(out=pi[:], in_=pb[:])
                pf = work.tile([P, 64], F32, tag="pf")
                nc.vector.tensor_copy(out=pf[:], in_=pi[:])
                wf = work.tile([P, 64], F32, tag="wf")
                nc.vector.tensor_tensor(out=wf[:], in0=pb[:], in1=pf[:], op=OP.subtract)
                neg = work.tile([P, 64], F32, tag="neg")
                nc.vector.tensor_scalar(out=neg[:], in0=wf[:], scalar1=0.0, scalar2=None,
                                        op0=OP.is_lt)
                nc.vector.tensor_tensor(out=wf[:], in0=wf[:], in1=neg[:], op=OP.add)
                x0 = work.tile([P, 64], F32, tag="x0")
                nc.vector.tensor_scalar_add(x0[:], pf[:], -1024.0)
                nc.vector.tensor_tensor(out=x0[:], in0=x0[:], in1=neg[:], op=OP.subtract)
                xs = work.tile([P, 64], F32, tag="xs")
                nc.vector.tensor_scalar(out=xs[:], in0=x0[:], scalar1=0.0, scalar2=63.0,
                                        op0=OP.max, op1=OP.min)
                eq0 = work.tile([P, 64], F32, tag="eq0")
                nc.vector.tensor_tensor(out=eq0[:], in0=xs[:], in1=x0[:], op=OP.is_equal)
                eqm1 = work.tile([P, 64], F32, tag="eqm1")
                nc.vector.tensor_scalar(out=eqm1[:], in0=x0[:], scalar1=-1.0, scalar2=None,
                                        op0=OP.is_equal)
                eq2 = work.tile([P, 64], F32, tag="eq2")
                nc.vector.tensor_scalar(out=eq2[:], in0=xs[:], scalar1=62.0, scalar2=None,
                                        op0=OP.min)
                nc.vector.tensor_tensor(out=eq2[:], in0=eq2[:], in1=x0[:], op=OP.is_equal)
                # lane weights: CL = (1-wf)*eq0 + wf*eqm1 ; CR = wf*eq2
                u = work.tile([P, 64], F32, tag="u")
                nc.vector.tensor_scalar(out=u[:], in0=wf[:], scalar1=1.0, scalar2=-1.0,
                                        op0=OP.subtract, op1=OP.mult)
                cl = work.tile([P, 64], F32, tag="cl")
                nc.vector.tensor_tensor(out=cl[:], in0=u[:], in1=eq0[:], op=OP.mult)
                tmp64 = work.tile([P, 64], F32, tag="tmp64")
                nc.vector.tensor_tensor(out=tmp64[:], in0=wf[:], in1=eqm1[:], op=OP.mult)
                nc.vector.tensor_tensor(out=cl[:], in0=cl[:], in1=tmp64[:], op=OP.add)
                cr = work.tile([P, 64], F32, tag="cr")
                nc.vector.tensor_tensor(out=cr[:], in0=wf[:], in1=eq2[:], op=OP.mult)

                # fold attention weight into the y-lane weights
                cl_y = cl[:].rearrange("p (a c) -> p a c", c=2)[:, :, 1]
                cr_y = cr[:].rearrange("p (a c) -> p a c", c=2)[:, :, 1]
                cl_x = cl[:].rearrange("p (a c) -> p a c", c=2)[:, :, 0]
                cr_x = cr[:].rearrange("p (a c) -> p a c", c=2)[:, :, 0]
                nc.vector.tensor_tensor(out=cl_y, in0=cl_y, in1=aw[:], op=OP.mult)
                nc.vector.tensor_tensor(out=cr_y, in0=cr_y, in1=aw[:], op=OP.mult)

                # W4[t][h,p,r,c] = Y_r * X_c
                w4t = w4_all[:, t]
                for r, yw in ((0, cl_y), (1, cr_y)):
                    for cc, xw in ((0, cl_x), (1, cr_x)):
                        nc.vector.tensor_tensor(
                            out=w4t[:, :, :, r, cc].rearrange("p h q -> p (h q)"),
                            in0=yw, in1=xw, op=OP.mult)

                # gather row index: (ys*64 + xs)*8 + h  (batch handled by
                # slicing the table, keeps values in int16 range)
                ys = xs[:].rearrange("p (a c) -> p a c", c=2)[:, :, 1]
                xs_x = xs[:].rearrange("p (a c) -> p a c", c=2)[:, :, 0]
                idxf = work.tile([P, 32], F32, tag="idxf")
                nc.vector.scalar_tensor_tensor(out=idxf[:], in0=ys, scalar=512.0,
                                               in1=hcst[:], op0=OP.mult, op1=OP.add)
                xs8 = work.tile([P, 32], F32, tag="xs8")
                nc.vector.tensor_scalar(out=xs8[:], in0=xs_x, scalar1=8.0,
                                        scalar2=None, op0=OP.mult)
                nc.vector.tensor_tensor(out=idxf[:], in0=idxf[:], in1=xs8[:], op=OP.add)
                # -> wrapped int16 layout via PE transpose + shuffled DRAM write
                tpi = ppt.tile([P, P], F32, tag="pt")
                nc.tensor.transpose(tpi[0:32, :], idxf[:], ident[:])
                # idx16[j, ql*8 + qh] = idx(q=qh*16+ql, j): the DVE copy
                # permutes q's subfields so both DMA sides stay contiguous
                idx16 = work.tile([32, P], I16, tag="idx16")
                nc.vector.tensor_copy(
                    out=idx16[:].rearrange("p (a c) -> p a c", a=16),
                    in_=tpi[0:32, :].rearrange("p (a c) -> p c a", a=8))
                nc.sync.dma_start(
                    out=bass.AP(tensor=idxshuf_d[:].tensor,
                                offset=(b * NT + t) * 4096,
                                ap=[[8, 32], [256, 16], [1, 8]]),
                    in_=idx16[:].rearrange("p (a c) -> p a c", a=16))

                # 4x shifted value projection -> patch rows for this tile.
                # Rows are d-major [DH, 4 corners] so the phase-B combine and
                # reduction run on contiguous innermost dims.
                h2sb = io.tile([P, NH, DH, 4], BF16, tag="h2sb")
                for ci, dlt in enumerate((0, 1, WS, WS + 1)):
                    vp = pps.tile([P, C], F32, tag="ps")
                    nc.tensor.matmul(vp[:], srcT[:, 0, t * P + dlt:t * P + dlt + P],
                                     wval[:, 0, :], start=True, stop=False)
                    nc.tensor.matmul(vp[:], srcT[:, 1, t * P + dlt:t * P + dlt + P],
                                     wval[:, 1, :], start=False, stop=True)
                    nc.vector.tensor_tensor(
                        out=h2sb[:, :, :, ci],
                        in0=vp[:].rearrange("p (h d) -> p h d", h=NH),
                        in1=bval[:].rearrange("p (h d) -> p h d", h=NH), op=OP.add)
                nc.sync.dma_start(
                    out=h2_d[(b * LQ + t * P) * NH:(b * LQ + (t + 1) * P) * NH, :],
                    in_=h2sb[:].rearrange("p h d c -> p (h d c)"))

            # -- gather indices for this batch into SBUF (8 Q7 core replicas)
            for g in range(8):
                nc.sync.dma_start(
                    out=idx_all[16 * g:16 * (g + 1), :, :],
                    in_=bass.AP(tensor=idxshuf_d[:].tensor, offset=b * NT * 4096,
                                ap=[[256, 16], [4096, NT], [1, 256]]))

            # ============ PHASE B+C per tile: gather, MSDA, out-proj, FFN ===
            h2b = h2_d[b * LQ * NH:(b + 1) * LQ * NH, :]
            for t in range(NT):
                r0 = b * LQ + t * P
                gt = work.tile([P, C], F32, tag="gt")  # MSDA output [q, (h d)]
                gat = patches_p.tile([P, 32, 4 * DH], BF16, tag="gat")
                for gq in range(4):
                    nc.gpsimd.dma_gather(
                        gat[:, gq * 8:(gq + 1) * 8, :], h2b,
                        idx_all[:, t, gq * 64:(gq + 1) * 64],
                        1024, 1024, 4 * DH)
                # weighted sum over (point, corner): one fused multiply over
                # all heads (contiguous), reduce corners (contiguous), then a
                # 3-op tree add over points
                mac = work.tile([P, 32, DH, 4], BF16, tag="mac")
                w4t = w4_all[:, t]
                w4v = bass.AP(tensor=w4t.tensor, offset=w4t.offset,
                              ap=[w4t.ap[0], [4, 32], [0, DH], [1, 4]])
                nc.vector.tensor_tensor(
                    out=mac[:],
                    in0=gat[:].rearrange("p j (d c) -> p j d c", c=4),
                    in1=w4v, op=OP.mult)
                mred = work.tile([P, 32 * DH], BF16, tag="mred")
                with nc.allow_low_precision("bf16 4-term corner sums, 2e-2 budget"):
                    nc.vector.reduce_sum(
                        out=mred[:],
                        in_=mac[:].rearrange("p j d c -> p (j d) c"),
                        axis=mybir.AxisListType.X)
                mv = mred[:].rearrange("p (h a d) -> p h a d", h=NH, a=NP)
                nc.vector.tensor_tensor(out=gt[:].rearrange("p (h d) -> p h d", h=NH),
                                        in0=mv[:, :, 0], in1=mv[:, :, 1], op=OP.add)
                nc.vector.tensor_tensor(out=gt[:].rearrange("p (h d) -> p h d", h=NH),
                                        in0=gt[:].rearrange("p (h d) -> p h d", h=NH),
                                        in1=mv[:, :, 2], op=OP.add)
                nc.vector.tensor_tensor(out=gt[:].rearrange("p (h d) -> p h d", h=NH),
                                        in0=gt[:].rearrange("p (h d) -> p h d", h=NH),
                                        in1=mv[:, :, 3], op=OP.add)

                # out projection needs G^T
                gT = work.tile([P, 2, P], BF16, tag="gT")
                for k in range(2):
                    tp = ppt.tile([P, P], F32, tag="pt")
                    nc.tensor.transpose(tp[:], gt[:, 128 * k:128 * (k + 1)], ident[:])
                    nc.scalar.copy(out=gT[:, k, :], in_=tp[:])
                ao = pps.tile([P, C], F32, tag="ps")
                nc.tensor.matmul(ao[:], gT[:, 0, :], wout[:, 0, :], start=True, stop=False)
                nc.tensor.matmul(ao[:], gT[:, 1, :], wout[:, 1, :], start=False, stop=True)

                # x1 = LN1(src + attn_out + b_out)
                src_t = io.tile([P, C], F32, tag="src_t2")
                nc.sync.dma_start(out=src_t[:], in_=src_d[r0:r0 + P, :])
                s1 = work.tile([P, C], F32, tag="s1")
                nc.vector.tensor_tensor(out=s1[:], in0=ao[:], in1=bout[:], op=OP.add)
                nc.vector.tensor_tensor(out=s1[:], in0=s1[:], in1=src_t[:], op=OP.add)
                x1 = work.tile([P, C], F32, tag="x1")
                _layernorm(nc, work, x1, s1, ln1g, ln1b)

                # FFN
                x1T = work.tile([P, 2, P], BF16, tag="x1T")
                for k in range(2):
                    tp = ppt.tile([P, P], F32, tag="pt")
                    nc.tensor.transpose(tp[:], x1[:, 128 * k:128 * (k + 1)], ident[:])
                    nc.scalar.copy(out=x1T[:, k, :], in_=tp[:])
                hT = work.tile([P, 8, P], BF16, tag="hT")
                for j in range(8):
                    fp = ppf.tile([P, P], F32, tag="pf")
                    nc.tensor.matmul(fp[:], w1[:, 0, 128 * j:128 * (j + 1)], x1T[:, 0, :],
                                     start=True, stop=False)
                    nc.tensor.matmul(fp[:], w1[:, 1, 128 * j:128 * (j + 1)], x1T[:, 1, :],
                                     start=False, stop=True)
                    nc.scalar.activation(out=hT[:, j, :], in_=fp[:], func=AF.Relu,
                                         bias=b1t[:, j:j + 1])
                y2 = pps.tile([P, C], F32, tag="ps")
                for j in range(8):
                    nc.tensor.matmul(y2[:], hT[:, j, :], w2[:, j, :],
                                     start=(j == 0), stop=(j == 7))
                s2 = work.tile([P, C], F32, tag="s2")
                nc.vector.tensor_tensor(out=s2[:], in0=y2[:], in1=b2r[:], op=OP.add)
                nc.vector.tensor_tensor(out=s2[:], in0=s2[:], in1=x1[:], op=OP.add)
                o_t = io.tile([P, C], F32, tag="o_t")
                _layernorm(nc, work, o_t, s2, ln2g, ln2b)
                nc.sync.dma_start(out=out_d[r0:r0 + P, :], in_=o_t[:])

    nc.compile()
    return nc


def _layernorm(nc, work, out_t, s, g_rep, b_rep, eps=1e-5):
    st6 = work.tile([P, 6], F32, tag="ln_st6")
    nc.vector.bn_stats(out=st6[:], in_=s[:])
    st2 = work.tile([P, 2], F32, tag="ln_st2")  # (mean, var)
    nc.vector.bn_aggr(out=st2[:], in_=st6[:])
    rstd = work.tile([P, 1], F32, tag="ln_rstd")
    nc.vector.tensor_scalar(out=rstd[:], in0=st2[:, 1:2], scalar1=eps, scalar2=None,
                            op0=OP.add)
    nc.vector.reciprocal(out=rstd[:], in_=rstd[:])
    nc.scalar.activation(out=rstd[:], in_=rstd[:], func=AF.Sqrt)
    negmr = work.tile([P, 1], F32, tag="ln_negmr")
    nc.vector.tensor_tensor(out=negmr[:], in0=st2[:, 0:1], in1=rstd[:], op=OP.mult)
    nc.vector.tensor_scalar(out=negmr[:], in0=negmr[:], scalar1=-1.0, scalar2=None,
                            op0=OP.mult)
    xn = work.tile([P, C], F32, tag="ln_xn")
    nc.scalar.activation(out=xn[:], in_=s[:], func=AF.Identity,
                         bias=negmr[:, 0:1], scale=rstd[:, 0:1])
    nc.vector.tensor_tensor(out=out_t[:], in0=xn[:], in1=g_rep[:], op=OP.mult)
    nc.vector.tensor_tensor(out=out_t[:], in0=out_t[:], in1=b_rep[:], op=OP.add)


def make_in_maps(inputs):
    src = np.ascontiguousarray(np.asarray(inputs["src"], dtype=np.float32))
    pos = np.ascontiguousarray(np.asarray(inputs["pos"], dtype=np.float32))
    ref = np.ascontiguousarray(np.asarray(inputs["reference_points"], dtype=np.float32))
    import ml_dtypes
    names = ["W_off", "b_off", "W_attn", "b_attn", "W_val", "b_val", "W_out",
             "b_out", "ln1_g", "ln1_b", "b1", "b2", "ln2_g", "ln2_b"]
    w = {n: np.ascontiguousarray(np.asarray(inputs[n], dtype=np.float32)) for n in names}
    for n in ("W1", "W2", "W_off", "W_attn", "W_val", "W_out"):
        w[n] = np.ascontiguousarray(
            np.asarray(inputs[n], dtype=np.float32).astype(ml_dtypes.bfloat16))

    in_maps = []
    for c in range(NCORES):
        m = dict(w)
        sc = src[BPC * c:BPC * (c + 1)]
        pc = pos[BPC * c:BPC * (c + 1)]
        m["src"] = sc.reshape(BPC * LQ, C)
        m["srcT"] = np.ascontiguousarray(
            sc.transpose(0, 2, 1).astype(ml_dtypes.bfloat16)).reshape(BPC * C, LQ)
        m["posT"] = np.ascontiguousarray(
            pc.transpose(0, 2, 1).astype(ml_dtypes.bfloat16)).reshape(BPC * C, LQ)
        m["refp"] = ref[BPC * c:BPC * (c + 1), :, 0, :].reshape(BPC * LQ, 2)
        in_maps.append(m)
    return in_maps


def assemble_output(results):
    out = np.stack([results[c]["out"].reshape(BPC, LQ, C) for c in range(NCORES)])
    return out.reshape(B, LQ, C)


def kernel(**inputs):
    if "nc" not in _CACHE:
        _CACHE["nc"] = build_nc()
    nc = _CACHE["nc"]
    in_maps = make_in_maps(inputs)
    res = run_bass_kernel_spmd(nc, in_maps, core_ids=list(range(NCORES)))
    return assemble_output(res.results)



# revision 28
# speedup vs baseline: 2.4276x; 2.4276x over previous
"""Deformable-attention encoder layer (single level) on 8 Trainium2 cores.

Data-parallel over batch: B=16 -> 2 images per NeuronCore. Each core runs the
full layer for its 2 images; no collectives.

MSDA sampling strategy: the value projection is computed 4 times with the src
rows shifted by (0, 1, 64, 65), giving, for every spatial cell e=(y,x), the
full 2x2 bilinear patch [v(y,x), v(y,x+1), v(y+1,x), v(y+1,x+1)] laid out
contiguously per head (128 bf16 = 256B). These patches are written to a DRAM
table indexed by (e, head), and fetched with gpsimd dma_gather in chunks of
1024 indices (>=2048 indices per gather overflows the SWDGE descriptor ring
on this runtime and wedges the device; per-partition-offset indirect DMAs
work but cost ~1us of Q7 time each, 2048 of them dominated the old runtime).
The int16 index tile must be in dma_gather's wrapped layout (index i at
partition i%16, column i//16, replicated across the 8 Q7 core groups), which
is produced by a PE transpose + a shuffled DRAM round trip. Bilinear corner
weights, OOB masks and attention weights are folded into 16 per-point
coefficients applied on the Vector engine, followed by a strided reduce.
"""

import sys
from contextlib import ExitStack

import numpy as np

sys.path.insert(0, "/opt/trn_rl_repo")

import concourse.bass as bass
import concourse.bacc as bacc
import concourse.mybir as mybir
import concourse.tile as tile
from concourse.bass_utils import run_bass_kernel_spmd

F32 = mybir.dt.float32
BF16 = mybir.dt.bfloat16
I16 = mybir.dt.int16
I32 = mybir.dt.int32
AF = mybir.ActivationFunctionType
OP = mybir.AluOpType

B, LQ, C = 16, 4096, 256
NH, NP, DH = 8, 4, 32
HS, WS = 64, 64
DFF = 1024
NCORES = 8
BPC = B // NCORES          # batches per core
P = 128
NT = LQ // P               # q-tiles per batch
EPAD = 128                 # srcT column padding for shifted windows

_CACHE = {}


def _bcast_ap(handle_or_ap, n_free):
    """DRAM [n] -> AP [128, n] replicated across partitions."""
    ap = handle_or_ap[:] if not isinstance(handle_or_ap, bass.AP) else handle_or_ap
    return bass.AP(tensor=ap.tensor, offset=ap.offset, ap=[[0, P], [1, n_free]])


def _expand_d(ap, d):
    """Append a 0-step broadcast dim of size d to an AP."""
    return bass.AP(tensor=ap.tensor, offset=ap.offset, ap=[*ap.ap, [0, d]])


def build_nc():
    nc = bacc.Bacc("TRN2")

    src_d = nc.dram_tensor("src", [BPC * LQ, C], F32, kind="ExternalInput")
    # host-pretransposed [batch*C, LQ] copies: feed the matmul lhsT layout
    # directly, removing all phase-A PE transposes
    srcT_d = nc.dram_tensor("srcT", [BPC * C, LQ], BF16, kind="ExternalInput")
    posT_d = nc.dram_tensor("posT", [BPC * C, LQ], BF16, kind="ExternalInput")
    ref_d = nc.dram_tensor("refp", [BPC * LQ, 2], F32, kind="ExternalInput")
    w_off_d = nc.dram_tensor("W_off", [C, 64], BF16, kind="ExternalInput")
    b_off_d = nc.dram_tensor("b_off", [64], F32, kind="ExternalInput")
    w_attn_d = nc.dram_tensor("W_attn", [C, 32], BF16, kind="ExternalInput")
    b_attn_d = nc.dram_tensor("b_attn", [32], F32, kind="ExternalInput")
    w_val_d = nc.dram_tensor("W_val", [C, C], BF16, kind="ExternalInput")
    b_val_d = nc.dram_tensor("b_val", [C], F32, kind="ExternalInput")
    w_out_d = nc.dram_tensor("W_out", [C, C], BF16, kind="ExternalInput")
    b_out_d = nc.dram_tensor("b_out", [C], F32, kind="ExternalInput")
    ln1g_d = nc.dram_tensor("ln1_g", [C], F32, kind="ExternalInput")
    ln1b_d = nc.dram_tensor("ln1_b", [C], F32, kind="ExternalInput")
    w1_d = nc.dram_tensor("W1", [C, DFF], BF16, kind="ExternalInput")
    b1_d = nc.dram_tensor("b1", [DFF], F32, kind="ExternalInput")
    w2_d = nc.dram_tensor("W2", [DFF, C], BF16, kind="ExternalInput")
    b2_d = nc.dram_tensor("b2", [C], F32, kind="ExternalInput")
    ln2g_d = nc.dram_tensor("ln2_g", [C], F32, kind="ExternalInput")
    ln2b_d = nc.dram_tensor("ln2_b", [C], F32, kind="ExternalInput")
    out_d = nc.dram_tensor("out", [BPC * LQ, C], F32, kind="ExternalOutput")

    # patch table: row (b*4096 + e)*8 + h ... laid out [b, e, h, 128], bf16
    h2_d = nc.dram_tensor("h2tab", [BPC * LQ * NH, 4 * DH], BF16, kind="Internal")
    # per-tile gather indices in dma_gather's wrapped layout, int16:
    # block (b, t) holds idx(q, j) at offset j*8 + (q%16)*256 + q//16
    idxshuf_d = nc.dram_tensor("idxshuf", [BPC * NT * P * 32], I16,
                               kind="Internal")

    ident_d = nc.inline_tensor(np.eye(P, dtype=np.float32), "identconst")
    # per-(h,p) head index constant, replicated over partitions
    hc = np.tile(np.repeat(np.arange(NH, dtype=np.float32), NP)[None, :], (P, 1))
    hc_d = nc.inline_tensor(hc, "hconst")

    with ExitStack() as ctx:
        tc = ctx.enter_context(tile.TileContext(nc))
        consts = ctx.enter_context(tc.tile_pool(name="consts", bufs=1))
        persist = ctx.enter_context(tc.tile_pool(name="persist", bufs=1))
        work = ctx.enter_context(tc.tile_pool(name="work", bufs=2))
        io = ctx.enter_context(tc.tile_pool(name="io", bufs=4))
        patches_p = ctx.enter_context(tc.tile_pool(name="patches", bufs=4))
        ppt = ctx.enter_context(tc.tile_pool(name="ppt", bufs=2, space="PSUM"))
        pps = ctx.enter_context(tc.tile_pool(name="pps", bufs=4, space="PSUM"))
        ppf = ctx.enter_context(tc.tile_pool(name="ppf", bufs=2, space="PSUM"))

        # ---- constants / weights into SBUF ----
        ident = consts.tile([P, P], F32)
        nc.sync.dma_start(out=ident[:], in_=ident_d[:, :])
        hcst = consts.tile([P, 32], F32)
        nc.sync.dma_start(out=hcst[:], in_=hc_d[:, :])

        wb = consts.tile([P, 2, 96], BF16)          # W_off|W_attn, 2 k-tiles
        for k in range(2):
            nc.sync.dma_start(out=wb[:, k, 0:64], in_=w_off_d[128 * k:128 * (k + 1), :])
            nc.sync.dma_start(out=wb[:, k, 64:96], in_=w_attn_d[128 * k:128 * (k + 1), :])
        wval = consts.tile([P, 2, C], BF16)
        wout = consts.tile([P, 2, C], BF16)
        for k in range(2):
            nc.sync.dma_start(out=wval[:, k, :], in_=w_val_d[128 * k:128 * (k + 1), :])
            nc.sync.dma_start(out=wout[:, k, :], in_=w_out_d[128 * k:128 * (k + 1), :])
        w1 = consts.tile([P, 2, DFF], BF16)
        for k in range(2):
            nc.sync.dma_start(out=w1[:, k, :], in_=w1_d[128 * k:128 * (k + 1), :])
        w2 = consts.tile([P, 8, C], BF16)
        for j in range(8):
            nc.sync.dma_start(out=w2[:, j, :], in_=w2_d[128 * j:128 * (j + 1), :])

        boffattn = consts.tile([P, 96], F32)
        nc.sync.dma_start(out=boffattn[:, 0:64], in_=_bcast_ap(b_off_d, 64))
        nc.sync.dma_start(out=boffattn[:, 64:96], in_=_bcast_ap(b_attn_d, 32))
        bval = consts.tile([P, C], F32)
        nc.sync.dma_start(out=bval[:], in_=_bcast_ap(b_val_d, C))
        bout = consts.tile([P, C], F32)
        nc.sync.dma_start(out=bout[:], in_=_bcast_ap(b_out_d, C))
        ln1g = consts.tile([P, C], F32)
        nc.sync.dma_start(out=ln1g[:], in_=_bcast_ap(ln1g_d, C))
        ln1b = consts.tile([P, C], F32)
        nc.sync.dma_start(out=ln1b[:], in_=_bcast_ap(ln1b_d, C))
        b2r = consts.tile([P, C], F32)
        nc.sync.dma_start(out=b2r[:], in_=_bcast_ap(b2_d, C))
        ln2g = consts.tile([P, C], F32)
        nc.sync.dma_start(out=ln2g[:], in_=_bcast_ap(ln2g_d, C))
        ln2b = consts.tile([P, C], F32)
        nc.sync.dma_start(out=ln2b[:], in_=_bcast_ap(ln2b_d, C))
        # b1 transposed: [128, 8] with b1t[p, j] = b1[128j + p]
        b1t = consts.tile([P, 8], F32)
        b1_ap = bass.AP(tensor=b1_d[:].tensor, offset=0, ap=[[1, P], [P, 8]])
        nc.sync.dma_start(out=b1t[:], in_=b1_ap)

        for b in range(BPC):
            # ============ PHASE A: projections, idx/weights, patch table ====
            srcT = persist.tile([P, 2, LQ + EPAD], BF16, tag="srcT")
            for k in range(2):
                nc.vector.memset(srcT[:, k, LQ:LQ + EPAD], 0.0)
            # per-batch tags: batch b+1's phase A can start while batch b's
            # phase B still reads these
            idx_all = persist.tile([P, NT, 256], I16, tag=f"idx_all{b}")
            w4_all = persist.tile([P, NT, NH, NP, 2, 2], BF16, tag=f"w4_all{b}")

            # A1: load the pretransposed src for this batch (shifted windows
            # in the value projection read up to 192 columns ahead).
            for k in range(2):
                nc.sync.dma_start(
                    out=srcT[:, k, 0:LQ],
                    in_=srcT_d[b * 2 * P + k * P:b * 2 * P + (k + 1) * P, :])

            for t in range(NT):
                r0 = b * LQ + t * P
                ref_t = io.tile([P, 2], F32, tag="ref_t")
                nc.sync.dma_start(out=ref_t[:], in_=ref_d[r0:r0 + P, :])

                posT = work.tile([P, 2, P], BF16, tag="posT")
                for k in range(2):
                    nc.sync.dma_start(
                        out=posT[:, k, :],
                        in_=bass.AP(tensor=posT_d[:, :].tensor,
                                    offset=(b * 2 * P + k * P) * LQ + t * P,
                                    ap=[[LQ, P], [1, P]]))

                # off|attn projection: q @ [W_off|W_attn] = (src+pos) @ Wb
                oa = pps.tile([P, 96], F32, tag="ps")
                nc.tensor.matmul(oa[:], srcT[:, 0, t * P:(t + 1) * P], wb[:, 0, :],
                                 start=True, stop=False)
                nc.tensor.matmul(oa[:], posT[:, 0, :], wb[:, 0, :], start=False, stop=False)
                nc.tensor.matmul(oa[:], srcT[:, 1, t * P:(t + 1) * P], wb[:, 1, :],
                                 start=False, stop=False)
                nc.tensor.matmul(oa[:], posT[:, 1, :], wb[:, 1, :], start=False, stop=True)

                # attention softmax over NP (logits are small; skip max-sub)
                aw_e = work.tile([P, 32], F32, tag="aw_e")
                nc.vector.tensor_tensor(out=aw_e[:], in0=oa[:, 64:96], in1=boffattn[:, 64:96],
                                        op=OP.add)
                nc.scalar.activation(out=aw_e[:], in_=aw_e[:], func=AF.Exp)
                aw_s = work.tile([P, 8], F32, tag="aw_s")
                nc.vector.reduce_sum(out=aw_s[:], in_=aw_e[:].rearrange("p (h q) -> p h q", h=8),
                                     axis=mybir.AxisListType.X)
                nc.vector.reciprocal(out=aw_s[:], in_=aw_s[:])
                aw = work.tile([P, 32], F32, tag="aw")
                nc.vector.tensor_tensor(
                    out=aw[:].rearrange("p (h q) -> p h q", h=8),
                    in0=aw_e[:].rearrange("p (h q) -> p h q", h=8),
                    in1=_expand_d(aw_s[:], NP), op=OP.mult)

                # sampling locations: px = off + b_off + (ref*64 - 0.5)
                refs = work.tile([P, 2], F32, tag="refs")
                nc.vector.tensor_scalar(out=refs[:], in0=ref_t[:], scalar1=64.0,
                                        scalar2=-0.5, op0=OP.mult, op1=OP.add)
                px = work.tile([P, 64], F32, tag="px")
                nc.vector.tensor_tensor(out=px[:], in0=oa[:, 0:64], in1=boffattn[:, 0:64],
                                        op=OP.add)
                refs_b = bass.AP(tensor=refs[:].tensor, offset=refs[:].offset,
                                 ap=[refs[:].ap[0], [0, 32], [1, 2]])
                nc.vector.tensor_tensor(out=px[:].rearrange("p (a c) -> p a c", c=2),
                                        in0=px[:].rearrange("p (a c) -> p a c", c=2),
                                        in1=refs_b, op=OP.add)
                # clip to [-2, 66], bias by 1024, floor via int cast + fixup
                pc = work.tile([P, 64], F32, tag="pc")
                nc.vector.tensor_scalar(out=pc[:], in0=px[:], scalar1=-2.0, scalar2=66.0,
                                        op0=OP.max, op1=OP.min)
                pb = work.tile([P, 64], F32, tag="pb")
                nc.vector.tensor_scalar_add(pb[:], pc[:], 1024.0)
                pi = work.tile([P, 64], I32, tag="pi")
                nc.vector.tensor_copy(out=pi[:], in_=pb[:])
                pf = work.tile([P, 64], F32, tag="pf")
                nc.vector.tensor_copy(out=pf[:], in_=pi[:])
                wf = work.tile([P, 64], F32, tag="wf")
                nc.vector.tensor_tensor(out=wf[:], in0=pb[:], in1=pf[:], op=OP.subtract)
                neg = work.tile([P, 64], F32, tag="neg")
                nc.vector.tensor_scalar(out=neg[:], in0=wf[:], scalar1=0.0, scalar2=None,
                                        op0=OP.is_lt)
                nc.vector.tensor_tensor(out=wf[:], in0=wf[:], in1=neg[:], op=OP.add)
                x0 = work.tile([P, 64], F32, tag="x0")
                nc.vector.tensor_scalar_add(x0[:], pf[:], -1024.0)
                nc.vector.tensor_tensor(out=x0[:], in0=x0[:], in1=neg[:], op=OP.subtract)
                xs = work.tile([P, 64], F32, tag="xs")
                nc.vector.tensor_scalar(out=xs[:], in0=x0[:], scalar1=0.0, scalar2=63.0,
                                        op0=OP.max, op1=OP.min)
                eq0 = work.tile([P, 64], F32, tag="eq0")
                nc.vector.tensor_tensor(out=eq0[:], in0=xs[:], in1=x0[:], op=OP.is_equal)
                eqm1 = work.tile([P, 64], F32, tag="eqm1")
                nc.vector.tensor_scalar(out=eqm1[:], in0=x0[:], scalar1=-1.0, scalar2=None,
                                        op0=OP.is_equal)
                eq2 = work.tile([P, 64], F32, tag="eq2")
                nc.vector.tensor_scalar(out=eq2[:], in0=xs[:], scalar1=62.0, scalar2=None,
                                        op0=OP.min)
                nc.vector.tensor_tensor(out=eq2[:], in0=eq2[:], in1=x0[:], op=OP.is_equal)
                # lane weights: CL = (1-wf)*eq0 + wf*eqm1 ; CR = wf*eq2
                u = work.tile([P, 64], F32, tag="u")
                nc.vector.tensor_scalar(out=u[:], in0=wf[:], scalar1=1.0, scalar2=-1.0,
                                        op0=OP.subtract, op1=OP.mult)
                cl = work.tile([P, 64], F32, tag="cl")
                nc.vector.tensor_tensor(out=cl[:], in0=u[:], in1=eq0[:], op=OP.mult)
                tmp64 = work.tile([P, 64], F32, tag="tmp64")
                nc.vector.tensor_tensor(out=tmp64[:], in0=wf[:], in1=eqm1[:], op=OP.mult)
                nc.vector.tensor_tensor(out=cl[:], in0=cl[:], in1=tmp64[:], op=OP.add)
                cr = work.tile([P, 64], F32, tag="cr")
                nc.vector.tensor_tensor(out=cr[:], in0=wf[:], in1=eq2[:], op=OP.mult)

                # fold attention weight into the y-lane weights
                cl_y = cl[:].rearrange("p (a c) -> p a c", c=2)[:, :, 1]
                cr_y = cr[:].rearrange("p (a c) -> p a c", c=2)[:, :, 1]
                cl_x = cl[:].rearrange("p (a c) -> p a c", c=2)[:, :, 0]
                cr_x = cr[:].rearrange("p (a c) -> p a c", c=2)[:, :, 0]
                nc.vector.tensor_tensor(out=cl_y, in0=cl_y, in1=aw[:], op=OP.mult)
                nc.vector.tensor_tensor(out=cr_y, in0=cr_y, in1=aw[:], op=OP.mult)

                # W4[t][h,p,r,c] = Y_r * X_c
                w4t = w4_all[:, t]
                for r, yw in ((0, cl_y), (1, cr_y)):
                    for cc, xw in ((0, cl_x), (1, cr_x)):
                        nc.vector.tensor_tensor(
                            out=w4t[:, :, :, r, cc].rearrange("p h q -> p (h q)"),
                            in0=yw, in1=xw, op=OP.mult)

                # gather row index: (ys*64 + xs)*8 + h  (batch handled by
                # slicing the table, keeps values in int16 range)
                ys = xs[:].rearrange("p (a c) -> p a c", c=2)[:, :, 1]
                xs_x = xs[:].rearrange("p (a c) -> p a c", c=2)[:, :, 0]
                idxf = work.tile([P, 32], F32, tag="idxf")
                nc.vector.scalar_tensor_tensor(out=idxf[:], in0=ys, scalar=512.0,
                                               in1=hcst[:], op0=OP.mult, op1=OP.add)
                xs8 = work.tile([P, 32], F32, tag="xs8")
                nc.vector.tensor_scalar(out=xs8[:], in0=xs_x, scalar1=8.0,
                                        scalar2=None, op0=OP.mult)
                nc.vector.tensor_tensor(out=idxf[:], in0=idxf[:], in1=xs8[:], op=OP.add)
                # -> wrapped int16 layout via PE transpose + shuffled DRAM write
                tpi = ppt.tile([P, P], F32, tag="pt")
                nc.tensor.transpose(tpi[0:32, :], idxf[:], ident[:])
                # idx16[j, ql*8 + qh] = idx(q=qh*16+ql, j): the DVE copy
                # permutes q's subfields so both DMA sides stay contiguous
                idx16 = work.tile([32, P], I16, tag="idx16")
                nc.vector.tensor_copy(
                    out=idx16[:].rearrange("p (a c) -> p a c", a=16),
                    in_=tpi[0:32, :].rearrange("p (a c) -> p c a", a=8))
                nc.sync.dma_start(
                    out=bass.AP(tensor=idxshuf_d[:].tensor,
                                offset=(b * NT + t) * 4096,
                                ap=[[8, 32], [256, 16], [1, 8]]),
                    in_=idx16[:].rearrange("p (a c) -> p a c", a=16))

                # 4x shifted value projection -> patch rows for this tile.
                # Rows are d-major [DH, 4 corners] so the phase-B combine and
                # reduction run on contiguous innermost dims.
                h2sb = io.tile([P, NH, DH, 4], BF16, tag="h2sb")
                for ci, dlt in enumerate((0, 1, WS, WS + 1)):
                    vp = pps.tile([P, C], F32, tag="ps")
                    nc.tensor.matmul(vp[:], srcT[:, 0, t * P + dlt:t * P + dlt + P],
                                     wval[:, 0, :], start=True, stop=False)
                    nc.tensor.matmul(vp[:], srcT[:, 1, t * P + dlt:t * P + dlt + P],
                                     wval[:, 1, :], start=False, stop=True)
                    nc.vector.tensor_tensor(
                        out=h2sb[:, :, :, ci],
                        in0=vp[:].rearrange("p (h d) -> p h d", h=NH),
                        in1=bval[:].rearrange("p (h d) -> p h d", h=NH), op=OP.add)
                nc.sync.dma_start(
                    out=h2_d[(b * LQ + t * P) * NH:(b * LQ + (t + 1) * P) * NH, :],
                    in_=h2sb[:].rearrange("p h d c -> p (h d c)"))

            # -- gather indices for this batch into SBUF (8 Q7 core replicas)
            for g in range(8):
                nc.sync.dma_start(
                    out=idx_all[16 * g:16 * (g + 1), :, :],
                    in_=bass.AP(tensor=idxshuf_d[:].tensor, offset=b * NT * 4096,
                                ap=[[256, 16], [4096, NT], [1, 256]]))

            # ============ PHASE B+C per tile: gather, MSDA, out-proj, FFN ===
            h2b = h2_d[b * LQ * NH:(b + 1) * LQ * NH, :]
            for t in range(NT):
                r0 = b * LQ + t * P
                gt = work.tile([P, C], F32, tag="gt")  # MSDA output [q, (h d)]
                gat = patches_p.tile([P, 32, 4 * DH], BF16, tag="gat")
                for gq in range(4):
                    nc.gpsimd.dma_gather(
                        gat[:, gq * 8:(gq + 1) * 8, :], h2b,
                        idx_all[:, t, gq * 64:(gq + 1) * 64],
                        1024, 1024, 4 * DH)
                # weighted sum over (point, corner): one fused multiply over
                # all heads (contiguous), reduce corners (contiguous), then a
                # 3-op tree add over points
                mac = work.tile([P, 32, DH, 4], BF16, tag="mac")
                w4t = w4_all[:, t]
                w4v = bass.AP(tensor=w4t.tensor, offset=w4t.offset,
                              ap=[w4t.ap[0], [4, 32], [0, DH], [1, 4]])
                nc.vector.tensor_tensor(
                    out=mac[:],
                    in0=gat[:].rearrange("p j (d c) -> p j d c", c=4),
                    in1=w4v, op=OP.mult)
                mred = work.tile([P, 32 * DH], BF16, tag="mred")
                with nc.allow_low_precision("bf16 4-term corner sums, 2e-2 budget"):
                    nc.vector.reduce_sum(
                        out=mred[:],
                        in_=mac[:].rearrange("p j d c -> p (j d) c"),
                        axis=mybir.AxisListType.X)
                mv = mred[:].rearrange("p (h a d) -> p h a d", h=NH, a=NP)
                nc.vector.tensor_tensor(out=gt[:].rearrange("p (h d) -> p h d", h=NH),
                                        in0=mv[:, :, 0], in1=mv[:, :, 1], op=OP.add)
                nc.vector.tensor_tensor(out=gt[:].rearrange("p (h d) -> p h d", h=NH),
                                        in0=gt[:].rearrange("p (h d) -> p h d", h=NH),
                                        in1=mv[:, :, 2], op=OP.add)
                nc.vector.tensor_tensor(out=gt[:].rearrange("p (h d) -> p h d", h=NH),
                                        in0=gt[:].rearrange("p (h d) -> p h d", h=NH),
                                        in1=mv[:, :, 3], op=OP.add)

                # out projection needs G^T
                gT = work.tile([P, 2, P], BF16, tag="gT")
                for k in range(2):
                    tp = ppt.tile([P, P], F32, tag="pt")
                    nc.tensor.transpose(tp[:], gt[:, 128 * k:128 * (k + 1)], ident[:])
                    nc.scalar.copy(out=gT[:, k, :], in_=tp[:])
                ao = pps.tile([P, C], F32, tag="ps")
                nc.tensor.matmul(ao[:], gT[:, 0, :], wout[:, 0, :], start=True, stop=False)
                nc.tensor.matmul(ao[:], gT[:, 1, :], wout[:, 1, :], start=False, stop=True)

                # x1 = LN1(src + attn_out + b_out)
                src_t = io.tile([P, C], F32, tag="src_t2")
                nc.sync.dma_start(out=src_t[:], in_=src_d[r0:r0 + P, :])
                s1 = work.tile([P, C], F32, tag="s1")
                nc.vector.tensor_tensor(out=s1[:], in0=ao[:], in1=bout[:], op=OP.add)
                nc.vector.tensor_tensor(out=s1[:], in0=s1[:], in1=src_t[:], op=OP.add)
                x1 = work.tile([P, C], F32, tag="x1")
                _layernorm(nc, work, x1, s1, ln1g, ln1b)

                # FFN
                x1T = work.tile([P, 2, P], BF16, tag="x1T")
                for k in range(2):
                    tp = ppt.tile([P, P], F32, tag="pt")
                    nc.tensor.transpose(tp[:], x1[:, 128 * k:128 * (k + 1)], ident[:])
                    nc.scalar.copy(out=x1T[:, k, :], in_=tp[:])
                hT = work.tile([P, 8, P], BF16, tag="hT")
                for j in range(8):
                    fp = ppf.tile([P, P], F32, tag="pf")
                    nc.tensor.matmul(fp[:], w1[:, 0, 128 * j:128 * (j + 1)], x1T[:, 0, :],
                                     start=True, stop=False)
                    nc.tensor.matmul(fp[:], w1[:, 1, 128 * j:128 * (j + 1)], x1T[:, 1, :],
                                     start=False, stop=True)
                    nc.scalar.activation(out=hT[:, j, :], in_=fp[:], func=AF.Relu,
                                         bias=b1t[:, j:j + 1])
                y2 = pps.tile([P, C], F32, tag="ps")
                for j in range(8):
                    nc.tensor.matmul(y2[:], hT[:, j, :], w2[:, j, :],
                                     start=(j == 0), stop=(j == 7))
                s2 = work.tile([P, C], F32, tag="s2")
                nc.vector.tensor_tensor(out=s2[:], in0=y2[:], in1=b2r[:], op=OP.add)
                nc.vector.tensor_tensor(out=s2[:], in0=s2[:], in1=x1[:], op=OP.add)
                o_t = io.tile([P, C], F32, tag="o_t")
                _layernorm(nc, work, o_t, s2, ln2g, ln2b)
                nc.sync.dma_start(out=out_d[r0:r0 + P, :], in_=o_t[:])

    nc.compile()
    return nc


def _layernorm(nc, work, out_t, s, g_rep, b_rep, eps=1e-5):
    st6 = work.tile([P, 6], F32, tag="ln_st6")
    nc.vector.bn_stats(out=st6[:], in_=s[:])
    st2 = work.tile([P, 2], F32, tag="ln_st2")  # (mean, var)
    nc.vector.bn_aggr(out=st2[:], in_=st6[:])
    rstd = work.tile([P, 1], F32, tag="ln_rstd")
    nc.vector.tensor_scalar(out=rstd[:], in0=st2[:, 1:2], scalar1=eps, scalar2=None,
                            op0=OP.add)
    nc.vector.reciprocal(out=rstd[:], in_=rstd[:])
    nc.scalar.activation(out=rstd[:], in_=rstd[:], func=AF.Sqrt)
    negmr = work.tile([P, 1], F32, tag="ln_negmr")
    nc.vector.tensor_tensor(out=negmr[:], in0=st2[:, 0:1], in1=rstd[:], op=OP.mult)
    nc.vector.tensor_scalar(out=negmr[:], in0=negmr[:], scalar1=-1.0, scalar2=None,
                            op0=OP.mult)
    xn = work.tile([P, C], F32, tag="ln_xn")
    nc.scalar.activation(out=xn[:], in_=s[:], func=AF.Identity,
                         bias=negmr[:, 0:1], scale=rstd[:, 0:1])
    nc.vector.tensor_tensor(out=out_t[:], in0=xn[:], in1=g_rep[:], op=OP.mult)
    nc.vector.tensor_tensor(out=out_t[:], in0=out_t[:], in1=b_rep[:], op=OP.add)


def make_in_maps(inputs):
    src = np.ascontiguousarray(np.asarray(inputs["src"], dtype=np.float32))
    pos = np.ascontiguousarray(np.asarray(inputs["pos"], dtype=np.float32))
    ref = np.ascontiguousarray(np.asarray(inputs["reference_points"], dtype=np.float32))
    import ml_dtypes
    names = ["W_off", "b_off", "W_attn", "b_attn", "W_val", "b_val", "W_out",
             "b_out", "ln1_g", "ln1_b", "b1", "b2", "ln2_g", "ln2_b"]
    w = {n: np.ascontiguousarray(np.asarray(inputs[n], dtype=np.float32)) for n in names}
    for n in ("W1", "W2", "W_off", "W_attn", "W_val", "W_out"):
        w[n] = np.ascontiguousarray(
            np.asarray(inputs[n], dtype=np.float32).astype(ml_dtypes.bfloat16))

    in_maps = []
    for c in range(NCORES):
        m = dict(w)
        sc = src[BPC * c:BPC * (c + 1)]
        pc = pos[BPC * c:BPC * (c + 1)]
        m["src"] = sc.reshape(BPC * LQ, C)
        m["srcT"] = np.ascontiguousarray(
            sc.transpose(0, 2, 1).astype(ml_dtypes.bfloat16)).reshape(BPC * C, LQ)
        m["posT"] = np.ascontiguousarray(
            pc.transpose(0, 2, 1).astype(ml_dtypes.bfloat16)).reshape(BPC * C, LQ)
        m["refp"] = ref[BPC * c:BPC * (c + 1), :, 0, :].reshape(BPC * LQ, 2)
        in_maps.append(m)
    return in_maps


def assemble_output(results):
    out = np.stack([results[c]["out"].reshape(BPC, LQ, C) for c in range(NCORES)])
    return out.reshape(B, LQ, C)


def kernel(**inputs):
    if "nc" not in _CACHE:
        _CACHE["nc"] = build_nc()
    nc = _CACHE["nc"]
    in_maps = make_in_maps(inputs)
    res = run_bass_kernel_spmd(nc, in_maps, core_ids=list(range(NCORES)))
    return assemble_output(res.results)

